# revision 5
# baseline (speedup 1.0000x reference)
"""3D Haar wavelet transform (2x2x2, causal temporal pad) on 8 Trainium2 cores.

Input  x: (2, 3, 33, 512, 512) fp32
Output y: (2, 24, 17, 256, 256) fp32   (channel = 3*s + c, s = subband)

Sharding: pure data parallel over H — core ci handles input rows
[64*ci, 64*ci+64) i.e. output rows [32*ci, 32*ci+32).

All three Haar stages (T, H, W) fold into ONE 128x128 matmul by putting
the three 2x2x2-block parities on the partition axis:
  input partition  p = i*64 + j*32 + k*16 + qlo
    (i = temporal offset, j = h parity, k = w parity, qlo = q mod 16
     where h = 2q + j, w = 2w' + k, q = qhi*16 + qlo)
  output partition m = di*64 + dj*32 + dw*16 + qlo, subband s = 4di+2dj+dw
  W[p, m] = (-1)^(i*di + j*dj + k*dw) iff qlo matches (8 nonzeros/col)
Free dim carries (qhi, T', w') = 2*17*256 = 8704 values per partition.

Everything runs in bf16 (the 2e-2 rel-err budget dwarfs bf16's ~2e-3):
host casts input, kernel writes bf16, host upcasts the gathered output.
That halves HBM traffic to 13.4MB in + 13.4MB out per core.

Per-core pipeline, per (b, c):
  1 in-DMA  [128, 8704] bf16 (2.23MB contiguous, SP HWDGE ring)
  17x matmul [128p, 512-chunk] -> PSUM fp32
  17x evacuate PSUM -> C bf16 with x0.3536, alternating ACT / DVE
  1 out-DMA [128, 8704] bf16 (2.23MB contiguous, ACT HWDGE ring)
Host reorders y' -> y (subband-major channels, h' concat) and upcasts.
"""

import numpy as np
import ml_dtypes

import concourse.bacc as bacc
import concourse.mybir as mybir
from concourse import tile
from concourse.bass_utils import run_bass_kernel_spmd

P = 128
B_, C_, T_, H_, W_ = 2, 3, 33, 512, 512
NCORES = 8
HC = H_ // NCORES          # 64 input rows per core
TP = (T_ + 1) // 2         # 17 output frames
HP = HC // 2               # 32 output rows per core
WP = W_ // 2               # 256 output cols
SCALE = float(np.float32(0.3536))
F32 = mybir.dt.float32
BF16 = mybir.dt.bfloat16
BF16_NP = ml_dtypes.bfloat16
FREE = 2 * TP * WP         # 8704 = (qhi, T', w') per partition per (b, c)
NCHUNK = FREE // 512       # 17 matmul chunks of 512


def _haar_matrix() -> np.ndarray:
    """W[p, m] with p = i*64+j*32+k*16+qlo, m = di*64+dj*32+dw*16+qlo."""
    W = np.zeros((P, P), dtype=np.float32)
    for i in range(2):
        for j in range(2):
            for k in range(2):
                for q in range(16):
                    p = i * 64 + j * 32 + k * 16 + q
                    for di in range(2):
                        for dj in range(2):
                            for dw in range(2):
                                m = di * 64 + dj * 32 + dw * 16 + q
                                W[p, m] = (-1.0) ** (i * di + j * dj + k * dw)
    return W.astype(BF16_NP)


def build_nc():
    nc = bacc.Bacc("TRN2", target_bir_lowering=False, debug=False)
    # x': [b, c, i, j, k, qlo, qhi, T', w'] host-pretransposed bf16
    x_d = nc.dram_tensor(
        "x", [B_, C_, 2, 2, 2, 16, 2, TP, WP], BF16, kind="ExternalInput"
    )
    # y': [b, c, m, qhi, T', w'] bf16
    y_d = nc.dram_tensor("y", [B_, C_, P, 2, TP, WP], BF16, kind="ExternalOutput")
    w_d = nc.inline_tensor(_haar_matrix(), name="haar_w")

    with tile.TileContext(nc) as tc:
        with (
            tc.tile_pool(name="wpool", bufs=1) as wpool,
            tc.tile_pool(name="apool", bufs=4) as apool,
            tc.tile_pool(name="cpool", bufs=4) as cpool,
            tc.tile_pool(name="psum", bufs=8, space="PSUM") as psum_pool,
        ):
            w_sb = wpool.tile([P, P], BF16)
            nc.gpsimd.dma_start(out=w_sb[:], in_=w_d[:])

            # in halves on sync (compute starts after the first half lands);
            # out thirds fan across all three rings: chunks 0-5 on the ACT
            # HWDGE ring, 6-11 on the gpsimd SWDGE ring, 12-16 on the sync
            # HWDGE ring (idle once the in-stream drains), so the tail of
            # the run drains the last tile on three rings concurrently.
            HI = 9 * 512
            O1, O2 = 6 * 512, 12 * 512
            for b in range(B_):
                for c in range(C_):
                    xin = x_d[b, c].rearrange("i j k q Q T w -> (i j k q) (Q T w)")
                    yout = y_d[b, c].rearrange("m Q T w -> m (Q T w)")
                    a = apool.tile([P, FREE], BF16, tag="a")
                    nc.sync.dma_start(out=a[:, 0:HI], in_=xin[:, 0:HI])
                    nc.sync.dma_start(out=a[:, HI:FREE], in_=xin[:, HI:FREE])
                    cbig = cpool.tile([P, FREE], BF16, tag="c")
                    for tg in range(NCHUNK):
                        sl = slice(tg * 512, (tg + 1) * 512)
                        ps = psum_pool.tile([P, 512], F32)
                        nc.tensor.matmul(
                            ps[:], w_sb[:], a[:, sl], start=True, stop=True
                        )
                        # evacuate + scale; alternate engines so neither
                        # ACT nor DVE exceeds the DMA budget
                        if tg % 2 == 0:
                            nc.scalar.mul(cbig[:, sl], ps[:], SCALE)
                        else:
                            nc.vector.tensor_scalar_mul(cbig[:, sl], ps[:], SCALE)
                        if tg == 5:
                            nc.scalar.dma_start(
                                out=yout[:, 0:O1], in_=cbig[:, 0:O1]
                            )
                        elif tg == 11:
                            nc.gpsimd.dma_start(
                                out=yout[:, O1:O2], in_=cbig[:, O1:O2]
                            )
                    nc.sync.dma_start(out=yout[:, O2:FREE], in_=cbig[:, O2:FREE])
    nc.compile()
    return nc


_NC_CACHE = None


def _get_nc():
    global _NC_CACHE
    if _NC_CACHE is None:
        _NC_CACHE = build_nc()
    return _NC_CACHE


# xp[tp] = x[max(tp-1, 0)] (causal pad); pair (T', i) reads xp[2T'+i]
_TIDX = np.maximum(np.arange(2 * TP) - 1, 0)


def _prep_core_input(xbf: np.ndarray, ci: int) -> np.ndarray:
    xc = xbf[:, :, _TIDX, HC * ci : HC * (ci + 1), :]    # [2,3,34,64,512] bf16
    # [b,c,T',i,(q,j)->h,(w',k)->w] split h and w into (quotient, parity)
    xc = xc.reshape(B_, C_, TP, 2, 2, 16, 2, WP, 2)      # [b,c,T',i,qh,ql,j,w',k]
    xc = xc.transpose(0, 1, 3, 6, 8, 5, 4, 2, 7)         # [b,c,i,j,k,ql,qh,T',w']
    return np.ascontiguousarray(xc)


def kernel(x: np.ndarray) -> np.ndarray:
    assert x.shape == (B_, C_, T_, H_, W_), x.shape
    xbf = np.asarray(x, dtype=np.float32).astype(BF16_NP)
    nc = _get_nc()
    in_maps = [{"x": _prep_core_input(xbf, ci)} for ci in range(NCORES)]
    res = run_bass_kernel_spmd(nc, in_maps, core_ids=list(range(NCORES)))
    y = np.empty((B_, 8 * C_, TP, H_ // 2, WP), dtype=np.float32)
    for ci in range(NCORES):
        yc = np.asarray(res.results[ci]["y"])            # [b,c,128,2,17,256] bf16
        yc = yc.reshape(B_, C_, 2, 2, 2, 16, 2, TP, WP)  # [b,c,di,dj,dw,ql,qh,T,w']
        yc = yc.transpose(0, 2, 3, 4, 1, 7, 6, 5, 8)     # [b,di,dj,dw,c,T,qh,ql,w']
        yc = yc.reshape(B_, 8 * C_, TP, HP, WP)          # ch = (4di+2dj+dw)*3+c
        y[:, :, :, HP * ci : HP * (ci + 1), :] = yc.astype(np.float32)
    return y


# revision 6
# speedup vs baseline: 1.0033x; 1.0033x over previous
"""3D Haar wavelet transform (2x2x2, causal temporal pad) on 8 Trainium2 cores.

Input  x: (2, 3, 33, 512, 512) fp32
Output y: (2, 24, 17, 256, 256) fp32   (channel = 3*s + c, s = subband)

Sharding: pure data parallel over H — core ci handles input rows
[64*ci, 64*ci+64) i.e. output rows [32*ci, 32*ci+32).

All three Haar stages (T, H, W) fold into ONE 128x128 matmul by putting
the three 2x2x2-block parities on the partition axis:
  input partition  p = i*64 + j*32 + k*16 + qlo
    (i = temporal offset, j = h parity, k = w parity, qlo = q mod 16
     where h = 2q + j, w = 2w' + k, q = qhi*16 + qlo)
  output partition m = di*64 + dj*32 + dw*16 + qlo, subband s = 4di+2dj+dw
  W[p, m] = (-1)^(i*di + j*dj + k*dw) iff qlo matches (8 nonzeros/col)
Free dim carries (qhi, T', w') = 2*17*256 = 8704 values per partition.

Everything runs in bf16 (the 2e-2 rel-err budget dwarfs bf16's ~2e-3):
host casts input, kernel writes bf16, host upcasts the gathered output.
That halves HBM traffic to 13.4MB in + 13.4MB out per core.

Per-core pipeline, per (b, c):
  1 in-DMA  [128, 8704] bf16 (2.23MB contiguous, SP HWDGE ring)
  17x matmul [128p, 512-chunk] -> PSUM fp32
  17x evacuate PSUM -> C bf16 with x0.3536, alternating ACT / DVE
  1 out-DMA [128, 8704] bf16 (2.23MB contiguous, ACT HWDGE ring)
Host reorders y' -> y (subband-major channels, h' concat) and upcasts.
"""

import numpy as np
import ml_dtypes

import concourse.bacc as bacc
import concourse.mybir as mybir
from concourse import tile
from concourse.bass_utils import run_bass_kernel_spmd

P = 128
B_, C_, T_, H_, W_ = 2, 3, 33, 512, 512
NCORES = 8
HC = H_ // NCORES          # 64 input rows per core
TP = (T_ + 1) // 2         # 17 output frames
HP = HC // 2               # 32 output rows per core
WP = W_ // 2               # 256 output cols
SCALE = float(np.float32(0.3536))
F32 = mybir.dt.float32
BF16 = mybir.dt.bfloat16
BF16_NP = ml_dtypes.bfloat16
FREE = 2 * TP * WP         # 8704 = (qhi, T', w') per partition per (b, c)
NCHUNK = FREE // 512       # 17 matmul chunks of 512


def _haar_matrix() -> np.ndarray:
    """W[p, m] with p = i*64+j*32+k*16+qlo, m = di*64+dj*32+dw*16+qlo."""
    W = np.zeros((P, P), dtype=np.float32)
    for i in range(2):
        for j in range(2):
            for k in range(2):
                for q in range(16):
                    p = i * 64 + j * 32 + k * 16 + q
                    for di in range(2):
                        for dj in range(2):
                            for dw in range(2):
                                m = di * 64 + dj * 32 + dw * 16 + q
                                W[p, m] = (-1.0) ** (i * di + j * dj + k * dw)
    return W.astype(BF16_NP)


def build_nc():
    nc = bacc.Bacc("TRN2", target_bir_lowering=False, debug=False)
    # x': [b, c, i, j, k, qlo, qhi, T', w'] host-pretransposed bf16
    x_d = nc.dram_tensor(
        "x", [B_, C_, 2, 2, 2, 16, 2, TP, WP], BF16, kind="ExternalInput"
    )
    # y': [b, c, m, qhi, T', w'] bf16
    y_d = nc.dram_tensor("y", [B_, C_, P, 2, TP, WP], BF16, kind="ExternalOutput")
    w_d = nc.inline_tensor(_haar_matrix(), name="haar_w")

    with tile.TileContext(nc) as tc:
        with (
            tc.tile_pool(name="wpool", bufs=1) as wpool,
            tc.tile_pool(name="apool", bufs=4) as apool,
            tc.tile_pool(name="cpool", bufs=4) as cpool,
            tc.tile_pool(name="psum", bufs=8, space="PSUM") as psum_pool,
        ):
            w_sb = wpool.tile([P, P], BF16)
            nc.scalar.dma_start(out=w_sb[:], in_=w_d[:])

            # The sync-ring FIFO carries ONLY the in-stream (a compute-
            # dependent out queued there would head-of-line-block future
            # ins).  Ins go as halves so compute starts after half a tile.
            # Outs: chunks 0-8 on the ACT HWDGE ring as soon as evacuated,
            # 9-16 on the gpsimd SWDGE ring.  The final tile fans its out
            # across all three rings (sync is idle after the last in).
            HI = 9 * 512
            O1, O2 = 6 * 512, 12 * 512
            NBC = B_ * C_
            for bc in range(NBC):
                b, c = divmod(bc, C_)
                last = bc == NBC - 1
                xin = x_d[b, c].rearrange("i j k q Q T w -> (i j k q) (Q T w)")
                yout = y_d[b, c].rearrange("m Q T w -> m (Q T w)")
                a = apool.tile([P, FREE], BF16, tag="a")
                nc.sync.dma_start(out=a[:, 0:HI], in_=xin[:, 0:HI])
                nc.sync.dma_start(out=a[:, HI:FREE], in_=xin[:, HI:FREE])
                cbig = cpool.tile([P, FREE], BF16, tag="c")
                for tg in range(NCHUNK):
                    sl = slice(tg * 512, (tg + 1) * 512)
                    ps = psum_pool.tile([P, 512], F32)
                    nc.tensor.matmul(
                        ps[:], w_sb[:], a[:, sl], start=True, stop=True
                    )
                    # evacuate + scale; alternate engines so neither
                    # ACT nor DVE exceeds the DMA budget
                    if tg % 2 == 0:
                        nc.vector.tensor_scalar_mul(cbig[:, sl], ps[:], SCALE)
                    else:
                        nc.scalar.mul(cbig[:, sl], ps[:], SCALE)
                    if not last:
                        if tg == 8:
                            nc.scalar.dma_start(
                                out=yout[:, 0:HI], in_=cbig[:, 0:HI]
                            )
                    else:
                        if tg == 5:
                            nc.scalar.dma_start(
                                out=yout[:, 0:O1], in_=cbig[:, 0:O1]
                            )
                        elif tg == 11:
                            nc.gpsimd.dma_start(
                                out=yout[:, O1:O2], in_=cbig[:, O1:O2]
                            )
                if not last:
                    nc.gpsimd.dma_start(out=yout[:, HI:FREE], in_=cbig[:, HI:FREE])
                else:
                    nc.sync.dma_start(out=yout[:, O2:FREE], in_=cbig[:, O2:FREE])
    nc.compile()
    return nc


_NC_CACHE = None


def _get_nc():
    global _NC_CACHE
    if _NC_CACHE is None:
        _NC_CACHE = build_nc()
    return _NC_CACHE


# xp[tp] = x[max(tp-1, 0)] (causal pad); pair (T', i) reads xp[2T'+i]
_TIDX = np.maximum(np.arange(2 * TP) - 1, 0)


def _prep_core_input(xbf: np.ndarray, ci: int) -> np.ndarray:
    xc = xbf[:, :, _TIDX, HC * ci : HC * (ci + 1), :]    # [2,3,34,64,512] bf16
    # [b,c,T',i,(q,j)->h,(w',k)->w] split h and w into (quotient, parity)
    xc = xc.reshape(B_, C_, TP, 2, 2, 16, 2, WP, 2)      # [b,c,T',i,qh,ql,j,w',k]
    xc = xc.transpose(0, 1, 3, 6, 8, 5, 4, 2, 7)         # [b,c,i,j,k,ql,qh,T',w']
    return np.ascontiguousarray(xc)


def kernel(x: np.ndarray) -> np.ndarray:
    assert x.shape == (B_, C_, T_, H_, W_), x.shape
    xbf = np.asarray(x, dtype=np.float32).astype(BF16_NP)
    nc = _get_nc()
    in_maps = [{"x": _prep_core_input(xbf, ci)} for ci in range(NCORES)]
    res = run_bass_kernel_spmd(nc, in_maps, core_ids=list(range(NCORES)))
    y = np.empty((B_, 8 * C_, TP, H_ // 2, WP), dtype=np.float32)
    for ci in range(NCORES):
        yc = np.asarray(res.results[ci]["y"])            # [b,c,128,2,17,256] bf16
        yc = yc.reshape(B_, C_, 2, 2, 2, 16, 2, TP, WP)  # [b,c,di,dj,dw,ql,qh,T,w']
        yc = yc.transpose(0, 2, 3, 4, 1, 7, 6, 5, 8)     # [b,di,dj,dw,c,T,qh,ql,w']
        yc = yc.reshape(B_, 8 * C_, TP, HP, WP)          # ch = (4di+2dj+dw)*3+c
        y[:, :, :, HP * ci : HP * (ci + 1), :] = yc.astype(np.float32)
    return y


# revision 7
# speedup vs baseline: 1.0176x; 1.0143x over previous
"""3D Haar wavelet transform (2x2x2, causal temporal pad) on 8 Trainium2 cores.

Input  x: (2, 3, 33, 512, 512) fp32
Output y: (2, 24, 17, 256, 256) fp32   (channel = 3*s + c, s = subband)

Sharding: pure data parallel over H — core ci handles input rows
[64*ci, 64*ci+64) i.e. output rows [32*ci, 32*ci+32).

All three Haar stages (T, H, W) fold into ONE 128x128 matmul by putting
the three 2x2x2-block parities on the partition axis:
  input partition  p = i*64 + j*32 + k*16 + qlo
    (i = temporal offset, j = h parity, k = w parity, qlo = q mod 16
     where h = 2q + j, w = 2w' + k, q = qhi*16 + qlo)
  output partition m = di*64 + dj*32 + dw*16 + qlo, subband s = 4di+2dj+dw
  W[p, m] = (-1)^(i*di + j*dj + k*dw) iff qlo matches (8 nonzeros/col)
Free dim carries (qhi, T', w') = 2*17*256 = 8704 values per partition.

Everything runs in bf16 (the 2e-2 rel-err budget dwarfs bf16's ~2e-3):
host casts input, kernel writes bf16, host upcasts the gathered output.
That halves HBM traffic to 13.4MB in + 13.4MB out per core.

Per-core pipeline, per (b, c):
  1 in-DMA  [128, 8704] bf16 (2.23MB contiguous, SP HWDGE ring)
  17x matmul [128p, 512-chunk] -> PSUM fp32
  17x evacuate PSUM -> C bf16 with x0.3536, alternating ACT / DVE
  1 out-DMA [128, 8704] bf16 (2.23MB contiguous, ACT HWDGE ring)
Host reorders y' -> y (subband-major channels, h' concat) and upcasts.
"""

import numpy as np
import ml_dtypes

import concourse.bacc as bacc
import concourse.mybir as mybir
from concourse import tile
from concourse.bass_utils import run_bass_kernel_spmd

P = 128
B_, C_, T_, H_, W_ = 2, 3, 33, 512, 512
NCORES = 8
HC = H_ // NCORES          # 64 input rows per core
TP = (T_ + 1) // 2         # 17 output frames
HP = HC // 2               # 32 output rows per core
WP = W_ // 2               # 256 output cols
SCALE = float(np.float32(0.3536))
F32 = mybir.dt.float32
BF16 = mybir.dt.bfloat16
BF16_NP = ml_dtypes.bfloat16
FREE = 2 * TP * WP         # 8704 = (qhi, T', w') per partition per (b, c)
NCHUNK = FREE // 512       # 17 matmul chunks of 512


def _haar_matrix() -> np.ndarray:
    """W[p, m] with p = i*64+j*32+k*16+qlo, m = di*64+dj*32+dw*16+qlo."""
    W = np.zeros((P, P), dtype=np.float32)
    for i in range(2):
        for j in range(2):
            for k in range(2):
                for q in range(16):
                    p = i * 64 + j * 32 + k * 16 + q
                    for di in range(2):
                        for dj in range(2):
                            for dw in range(2):
                                m = di * 64 + dj * 32 + dw * 16 + q
                                W[p, m] = (-1.0) ** (i * di + j * dj + k * dw)
    return W.astype(BF16_NP)


def build_nc():
    nc = bacc.Bacc("TRN2", target_bir_lowering=False, debug=False)
    # x': [b, c, i, j, k, qlo, qhi, T', w'] host-pretransposed bf16
    x_d = nc.dram_tensor(
        "x", [B_, C_, 2, 2, 2, 16, 2, TP, WP], BF16, kind="ExternalInput"
    )
    # y': [b, c, m, qhi, T', w'] bf16
    y_d = nc.dram_tensor("y", [B_, C_, P, 2, TP, WP], BF16, kind="ExternalOutput")
    w_d = nc.inline_tensor(_haar_matrix(), name="haar_w")

    with tile.TileContext(nc) as tc:
        with (
            tc.tile_pool(name="wpool", bufs=1) as wpool,
            tc.tile_pool(name="apool", bufs=4) as apool,
            tc.tile_pool(name="cpool", bufs=5) as cpool,
            tc.tile_pool(name="psum", bufs=4, space="PSUM") as psum_pool,
        ):
            w_sb = wpool.tile([P, P], BF16)
            nc.scalar.dma_start(out=w_sb[:], in_=w_d[:])

            # The sync-ring FIFO carries ONLY the in-stream (a compute-
            # dependent out queued there would head-of-line-block future
            # ins).  Ins go as halves so compute starts after half a tile.
            # Matmuls run per 512-chunk (PSUM bank limit) but evacuate in
            # 1024-wide pair ops alternating ACT / DVE.  Outs leave in three
            # ~0.74MB pieces alternating over the ACT-HWDGE and gpsimd-SWDGE
            # rings; the final tile also uses the (now idle) sync ring.
            HI = 9 * 512
            O1, O2 = 6 * 512, 12 * 512
            NBC = B_ * C_
            for bc in range(NBC):
                b, c = divmod(bc, C_)
                last = bc == NBC - 1
                xin = x_d[b, c].rearrange("i j k q Q T w -> (i j k q) (Q T w)")
                yout = y_d[b, c].rearrange("m Q T w -> m (Q T w)")
                a = apool.tile([P, FREE], BF16, tag="a")
                nc.sync.dma_start(out=a[:, 0:HI], in_=xin[:, 0:HI])
                nc.sync.dma_start(out=a[:, HI:FREE], in_=xin[:, HI:FREE])
                cbig = cpool.tile([P, FREE], BF16, tag="c")
                for pg in range(9):          # 8 chunk-pairs + final single
                    lo = pg * 1024
                    hi = min(lo + 1024, FREE)
                    ps = psum_pool.tile([P, 1024], F32)
                    for half in range((hi - lo) // 512):
                        s0 = lo + half * 512
                        nc.tensor.matmul(
                            ps[:, half * 512 : (half + 1) * 512],
                            w_sb[:],
                            a[:, s0 : s0 + 512],
                            start=True,
                            stop=True,
                        )
                    dst = cbig[:, lo:hi]
                    src = ps[:, 0 : hi - lo]
                    if pg % 2 == 0:
                        nc.vector.tensor_scalar_mul(dst, src, SCALE)
                    else:
                        nc.scalar.mul(dst, src, SCALE)
                    # out pieces: chunks 0-5 / 6-11 / 12-16, i.e. after
                    # pair-groups 2 (lo=2048..3071), 5, 8
                    if pg == 2:
                        eng = nc.scalar if bc % 2 == 0 else nc.gpsimd
                        eng.dma_start(out=yout[:, 0:O1], in_=cbig[:, 0:O1])
                    elif pg == 5:
                        eng = nc.gpsimd if bc % 2 == 0 else nc.scalar
                        eng.dma_start(out=yout[:, O1:O2], in_=cbig[:, O1:O2])
                if last:
                    nc.sync.dma_start(out=yout[:, O2:FREE], in_=cbig[:, O2:FREE])
                else:
                    eng = nc.scalar if bc % 2 == 0 else nc.gpsimd
                    eng.dma_start(out=yout[:, O2:FREE], in_=cbig[:, O2:FREE])
    nc.compile()
    return nc


_NC_CACHE = None


def _get_nc():
    global _NC_CACHE
    if _NC_CACHE is None:
        _NC_CACHE = build_nc()
    return _NC_CACHE


# xp[tp] = x[max(tp-1, 0)] (causal pad); pair (T', i) reads xp[2T'+i]
_TIDX = np.maximum(np.arange(2 * TP) - 1, 0)


def _prep_core_input(xbf: np.ndarray, ci: int) -> np.ndarray:
    xc = xbf[:, :, _TIDX, HC * ci : HC * (ci + 1), :]    # [2,3,34,64,512] bf16
    # [b,c,T',i,(q,j)->h,(w',k)->w] split h and w into (quotient, parity)
    xc = xc.reshape(B_, C_, TP, 2, 2, 16, 2, WP, 2)      # [b,c,T',i,qh,ql,j,w',k]
    xc = xc.transpose(0, 1, 3, 6, 8, 5, 4, 2, 7)         # [b,c,i,j,k,ql,qh,T',w']
    return np.ascontiguousarray(xc)


def kernel(x: np.ndarray) -> np.ndarray:
    assert x.shape == (B_, C_, T_, H_, W_), x.shape
    xbf = np.asarray(x, dtype=np.float32).astype(BF16_NP)
    nc = _get_nc()
    in_maps = [{"x": _prep_core_input(xbf, ci)} for ci in range(NCORES)]
    res = run_bass_kernel_spmd(nc, in_maps, core_ids=list(range(NCORES)))
    y = np.empty((B_, 8 * C_, TP, H_ // 2, WP), dtype=np.float32)
    for ci in range(NCORES):
        yc = np.asarray(res.results[ci]["y"])            # [b,c,128,2,17,256] bf16
        yc = yc.reshape(B_, C_, 2, 2, 2, 16, 2, TP, WP)  # [b,c,di,dj,dw,ql,qh,T,w']
        yc = yc.transpose(0, 2, 3, 4, 1, 7, 6, 5, 8)     # [b,di,dj,dw,c,T,qh,ql,w']
        yc = yc.reshape(B_, 8 * C_, TP, HP, WP)          # ch = (4di+2dj+dw)*3+c
        y[:, :, :, HP * ci : HP * (ci + 1), :] = yc.astype(np.float32)
    return y
